# revision 29
# baseline (speedup 1.0000x reference)
"""Trainium2 Bass kernel for an LSTM cell (DPLSTMCell).

  gates = input @ W_ih^T + b_ih + h_0 @ W_hh^T + b_hh          [B, 4H]
  i, f, g, o = split(gates, 4)
  c_1 = sigmoid(f) * c_0 + sigmoid(i) * tanh(g)
  h_1 = sigmoid(o) * tanh(c_1)

B=16384, IN=H=1024. Data-parallel across 8 NeuronCores: each core gets a
2048-row batch shard; weights/biases are replicated.

Mixed-precision matmuls: the sigmoid gates (i, f, o) tolerate fp8 with the
2e-2 relative-error budget, so their GEMMs run as fp8e4m3 DoubleRow
(double-pumped, 2x bf16 throughput); the tanh cell gate g dominates the
quantization-error budget and stays mostly bf16 — 2 of its 8 DoubleRow
k-pairs are offloaded to fp8 (split-K), spending the remaining error
budget for ~14us (measured rel_err 1.811e-2 vs 1.554e-2 full-bf16-g).
Weights for fp8 GEMMs are pre-scaled by 8 on the host to lift them out of
e4m3's subnormal range (W is uniform +-1/32); the inverse 1/8 rides the
ACT engine's free input scale, with the bias pre-scaled by 8 so the DVE
bias-add stays a plain add. The g gate's bf16 weights are also scaled by
8 (free for bf16) so both its halves accumulate consistently in PSUM.

Two scheduling tricks worth ~37us together (519 -> 253us journey:
290 fp8 / 283 dedupe / 253 drain-order):
 - Weight DMA arrives as kt-slabs spanning all fp8 gates, so the
   sim-driven Tile scheduler interleaves the PSUM banks kt-major; the
   _dedupe_ldweights pass then drops the now-adjacent duplicate
   LdWeights (the PE array holds weights across matmuls).
 - The epilogue drains PSUM banks in readiness order (0,1,3 close at the
   last fp8 h-wave and drain during g's bf16 tail; g closes at group end
   and drains last). DVE executes in order, so draining g first would
   gate every bank's release on the group's final matmul and stall the
   next group's first wave.

Host-side prep (inside kernel()): x^T / h^T tiles and W^T in a
matmul-friendly 5D layout, cast per-gate to bf16 or fp8; b_ih+b_hh
combined (fp8 gates' entries x8). Device: bf16/fp8 matmuls with fp32 PSUM
accumulation plus an fp32 sigmoid/tanh epilogue.

Device layout per core (q8 indexes fp8 gates, qb bf16 gates):
  xT8/hT8 [128, MT, KT, 128] fp8  : xT8[p, m, kt, b] = x[m*128 + b, kt*128 + p]
  xTb/hTb [128, MT, KT, 128] bf16 : same indexing (only if bf16 gates exist)
  w8ih/w8hh [128, KT, 2, n8, 512] fp8  : [p, kt, j, q8, s] = 8*W[g(q8)*1024 + j*512 + s, kt*128 + p]
  wbih/wbhh [128, KT, 2, nb, 512] bf16 : [p, kt, j, qb, s] =   W[g(qb)*1024 + j*512 + s, kt*128 + p]
  bias [1, 2, 4, 512] fp32 : bias[0, j, q, s] = scale_q * (b_ih + b_hh)[q*1024 + j*512 + s]
  c0 / h1 / c1 [2048, 1024] fp32 natural.

Per batch-tile m (128 rows) and gate-column group j (512 of 1024 columns):
4 PSUM banks (i, f, g, o); fp8 banks accumulate 8 DoubleRow matmuls of
[128k x 2 x 128b]^T @ [128k x 2 x 512g], the bf16 bank 16 matmuls of
[128k x 128b]^T @ [128k x 512g]; the fp32 bias (DMA-broadcast across
partitions once) is added on DVE during the PSUM->SBUF move, then ACT
applies sigmoid/tanh (with the 1/8 de-scale for fp8 gates) and DVE forms
c_1 / h_1.
"""

import os
import sys

import numpy as np

for _p in ("/opt/trn_rl_repo", "/root/.axon_site/_ro/trn_rl_repo"):
    if os.path.isdir(_p) and _p not in sys.path:
        sys.path.append(_p)

import ml_dtypes  # noqa: E402

import concourse.bass as bass  # noqa: E402
import concourse.mybir as mybir  # noqa: E402
import concourse.tile as tile  # noqa: E402
from concourse.bass_utils import run_bass_kernel_spmd  # noqa: E402

N_CORES = 8
B = 16384
IN = 1024
H = 1024
BL = B // N_CORES  # 2048 rows per core
MT = BL // 128     # 16 batch tiles per core
KT = IN // 128     # 8 k-tiles
NQ = 512           # free dim per PSUM bank
BF16 = ml_dtypes.bfloat16
FP8 = ml_dtypes.float8_e4m3  # TRN2 fp8e4 (max 240)

_GATE_IDX = {"i": 0, "f": 1, "g": 2, "o": 3}


def _fp8_gates():
    """Set of gate indices computed in fp8 (default: i, f, o; g stays bf16)."""
    s = os.environ.get("LSTM_FP8_GATES", "i,f,o")
    return frozenset(_GATE_IDX[t] for t in s.split(",") if t)


def _g_fp8_pairs():
    """Number of the g gate's 8 DoubleRow k-pairs offloaded to fp8 (split
    evenly between the x and h sides; the rest stay bf16). Each pair moves
    1/8 of g's contraction to fp8: err grows ~sqrt(1 + 0.6*pairs),
    stream shrinks 6.8us/pair."""
    return int(os.environ.get("LSTM_G_FP8_PAIRS", "2"))


def _wscale():
    return float(os.environ.get("LSTM_WSCALE", "8"))


# The walrus in this container only accepts one sync-wait command per
# instruction; Tile emits instructions (notably the final drain) with more.
_MAX_WAITS_PER_INST = 1


def _split_excess_waits(nc, cap=_MAX_WAITS_PER_INST):
    """Move excess sem-waits onto NoOps inserted ahead of the instruction
    (same engine). Waits are AND-conditions on monotonically increasing
    semaphores, so satisfying them one-by-one is equivalent."""
    for f in nc.m.functions:
        for blk in f.blocks:
            new_insts = []
            for inst in blk.instructions:
                si = getattr(inst, "sync_info", None)
                if si is not None and si.on_wait and len(si.on_wait) > cap:
                    waits = list(si.on_wait)
                    extra, keep = waits[:-cap], waits[-cap:]
                    while extra:
                        chunk, extra = extra[:cap], extra[cap:]
                        new_insts.append(
                            mybir.InstNoOp(
                                name=nc.get_next_instruction_name(),
                                sync_info=mybir.SyncInfo(on_wait=chunk, on_update=[]),
                                bass_nofuse=True,
                                engine=inst.engine,
                            )
                        )
                    inst.sync_info = mybir.SyncInfo(
                        on_wait=keep, on_update=list(si.on_update or [])
                    )
                new_insts.append(inst)
            blk.instructions[:] = new_insts


def _build_nc(repeat=None):
    """repeat>1 wraps the whole body in a hardware loop — benchmarking only
    (outputs are simply rewritten each iteration)."""
    if repeat is None:
        repeat = int(os.environ.get("LSTM_BENCH_REPEAT", "1"))
    fp8_gates = sorted(_fp8_gates())
    bf_gates = sorted(set(range(4)) - set(fp8_gates))
    n8, nb = len(fp8_gates), len(bf_gates)
    ws = _wscale()
    gp = _g_fp8_pairs() if 2 in bf_gates else 0
    gxp, ghp = (gp + 1) // 2, gp // 2  # g's fp8 k-pairs on the x / h side
    assert gp == 0 or n8, "split-K g needs the fp8 x/h tiles"
    # per-gate pre-activation scale (weights+bias scaled up by s, ACT
    # de-scales by 1/s): fp8 gates use ws to clear e4m3's subnormal range;
    # a split-K g uses ws on BOTH its fp8 and bf16 parts so the PSUM
    # accumulation is consistent (bf16 is scale-invariant, so this is free)
    gate_scale = [ws if (q in fp8_gates or (q == 2 and gp)) else 1.0
                  for q in range(4)]

    nc = bass.Bass()
    f32 = mybir.dt.float32
    bf16 = mybir.dt.bfloat16
    fp8 = mybir.dt.float8e4
    DR = mybir.MatmulPerfMode.DoubleRow
    SIG = mybir.ActivationFunctionType.Sigmoid
    TANH = mybir.ActivationFunctionType.Tanh
    ACT_FN = {0: SIG, 1: SIG, 2: TANH, 3: SIG}

    c0 = nc.declare_dram_parameter("c0", [BL, H], f32, isOutput=False)
    if n8:
        xT8 = nc.declare_dram_parameter("xT8", [128, MT, KT, 128], fp8, isOutput=False)
        hT8 = nc.declare_dram_parameter("hT8", [128, MT, KT, 128], fp8, isOutput=False)
        w8ih = nc.declare_dram_parameter("w8ih", [128, KT, 2, n8, NQ], fp8, isOutput=False)
        w8hh = nc.declare_dram_parameter("w8hh", [128, KT, 2, n8, NQ], fp8, isOutput=False)
    if gxp:
        w8gih = nc.declare_dram_parameter("w8gih", [128, 2 * gxp, 2, NQ], fp8, isOutput=False)
    if ghp:
        w8ghh = nc.declare_dram_parameter("w8ghh", [128, 2 * ghp, 2, NQ], fp8, isOutput=False)
    if nb:
        xTb = nc.declare_dram_parameter("xTb", [128, MT, KT, 128], bf16, isOutput=False)
        hTb = nc.declare_dram_parameter("hTb", [128, MT, KT, 128], bf16, isOutput=False)
        wbih = nc.declare_dram_parameter("wbih", [128, KT, 2, nb, NQ], bf16, isOutput=False)
        wbhh = nc.declare_dram_parameter("wbhh", [128, KT, 2, nb, NQ], bf16, isOutput=False)
    bjqs = nc.declare_dram_parameter("bjqs", [1, 2, 4, NQ], f32, isOutput=False)
    h1 = nc.declare_dram_parameter("h1", [BL, H], f32, isOutput=True)
    c1 = nc.declare_dram_parameter("c1", [BL, H], f32, isOutput=True)

    with tile.TileContext(nc) as tc:
        with (
            tc.tile_pool(name="w", bufs=1) as wpool,
            tc.tile_pool(name="xh", bufs=4) as xhpool,
            tc.tile_pool(name="cc", bufs=4) as cpool,
            tc.tile_pool(name="act", bufs=2) as apool,
            tc.tile_pool(name="outp", bufs=4) as opool,
            tc.tile_pool(name="ps", bufs=8, space="PSUM") as pspool,
        ):
            if n8:
                w8ih_sb = wpool.tile([128, KT, 2, n8, NQ], fp8)
                w8hh_sb = wpool.tile([128, KT, 2, n8, NQ], fp8)
            if gxp:
                w8gih_sb = wpool.tile([128, 2 * gxp, 2, NQ], fp8)
            if ghp:
                w8ghh_sb = wpool.tile([128, 2 * ghp, 2, NQ], fp8)
            if nb:
                wbih_sb = wpool.tile([128, KT, 2, nb, NQ], bf16)
                wbhh_sb = wpool.tile([128, KT, 2, nb, NQ], bf16)
            bias_sb = wpool.tile([128, 2, 4, NQ], f32)

            if repeat > 1:
                loop_cm = tc.For_i(0, repeat, 1)
                loop_cm.__enter__()

            # Weights on the SP HWDGE queue in ~consumption order, as
            # kt-slabs spanning ALL gates of a dtype class: with every
            # bank's kt-wave arriving together, the greedy scheduler
            # interleaves banks kt-major (priority = source order), which
            # makes the per-pair LdWeights shareable across banks — the
            # dedupe pass then drops the redundant reloads. x/h/c0 loads
            # go on the ACT queue; outputs on SP after weights.
            for j in range(2):
                if n8:
                    for kh in range(2):
                        ks = slice(kh * 4, (kh + 1) * 4)
                        nc.sync.dma_start(out=w8ih_sb[:, ks, j], in_=w8ih[:, ks, j])
                        if kh == 0 and gxp:
                            nc.sync.dma_start(out=w8gih_sb[:, :, j], in_=w8gih[:, :, j])
                    for kh in range(2):
                        ks = slice(kh * 4, (kh + 1) * 4)
                        nc.sync.dma_start(out=w8hh_sb[:, ks, j], in_=w8hh[:, ks, j])
                        if kh == 0 and ghp:
                            nc.sync.dma_start(out=w8ghh_sb[:, :, j], in_=w8ghh[:, :, j])
                if nb:
                    for w_sb, w_dr, gskip in (
                        (wbih_sb, wbih, 2 * gxp), (wbhh_sb, wbhh, 2 * ghp)
                    ):
                        for qi, q in enumerate(bf_gates):
                            lo = gskip if q == 2 else 0
                            for kh in range(2):
                                ks = slice(max(kh * 4, lo), (kh + 1) * 4)
                                if ks.start >= ks.stop:
                                    continue
                                nc.sync.dma_start(
                                    out=w_sb[:, ks, j, qi], in_=w_dr[:, ks, j, qi]
                                )

            for m in range(MT):
                if n8:
                    xm8 = xhpool.tile([128, KT, 128], fp8, tag="xm8")
                    hm8 = xhpool.tile([128, KT, 128], fp8, tag="hm8")
                    nc.scalar.dma_start(out=xm8, in_=xT8[:, m])
                    nc.scalar.dma_start(out=hm8, in_=hT8[:, m])
                if nb:
                    xmb = xhpool.tile([128, KT, 128], bf16, tag="xmb")
                    hmb = xhpool.tile([128, KT, 128], bf16, tag="hmb")
                    nc.scalar.dma_start(out=xmb, in_=xTb[:, m])
                    nc.scalar.dma_start(out=hmb, in_=hTb[:, m])
                if m == 0:
                    # bias isn't needed until the first matmul group finishes;
                    # keep it behind the first x/h tiles on the ACT queue.
                    bj_ap = bjqs[:]
                    bias_bcast = bass.AP(
                        tensor=bj_ap.tensor,
                        offset=bj_ap.offset,
                        ap=[[0, 128]] + list(bj_ap.ap[1:]),
                    )
                    nc.scalar.dma_start(out=bias_sb, in_=bias_bcast)
                for j in range(2):
                    cs = slice(j * NQ, (j + 1) * NQ)
                    rs = slice(m * 128, (m + 1) * 128)

                    c0t = cpool.tile([128, NQ], f32, tag="c0")
                    nc.scalar.dma_start(out=c0t, in_=c0[rs, cs])

                    ps = [
                        pspool.tile([128, NQ], f32, tag="ps", name=f"ps{q}")
                        for q in range(4)
                    ]
                    # fp8 gates: DoubleRow pairs two k-tiles per matmul. The
                    # g gate's split-K fp8 pairs ride the first kt-waves and
                    # share the interleaved chains' LdWeights.
                    for t in range(KT // 2):
                        kp = slice(2 * t, 2 * t + 2)
                        for qi, q in enumerate(fp8_gates):
                            nc.tensor.matmul(
                                ps[q], lhsT=xm8[:, kp], rhs=w8ih_sb[:, kp, j, qi],
                                start=(t == 0), stop=False,
                                perf_mode=DR, skip_group_check=True,
                            )
                        if t < gxp:
                            nc.tensor.matmul(
                                ps[2], lhsT=xm8[:, kp],
                                rhs=w8gih_sb[:, 2 * t:2 * t + 2, j],
                                start=(t == 0), stop=False,
                                perf_mode=DR, skip_group_check=True,
                            )
                    for t in range(KT // 2):
                        kp = slice(2 * t, 2 * t + 2)
                        last = t == KT // 2 - 1
                        for qi, q in enumerate(fp8_gates):
                            nc.tensor.matmul(
                                ps[q], lhsT=hm8[:, kp], rhs=w8hh_sb[:, kp, j, qi],
                                start=False, stop=last,
                                perf_mode=DR, skip_group_check=True,
                            )
                        if t < ghp:
                            nc.tensor.matmul(
                                ps[2], lhsT=hm8[:, kp],
                                rhs=w8ghh_sb[:, 2 * t:2 * t + 2, j],
                                start=False, stop=False,
                                perf_mode=DR, skip_group_check=True,
                            )
                    # bf16 gates: one k-tile per matmul (g skips its fp8 kts).
                    for kt in range(KT):
                        for qi, q in enumerate(bf_gates):
                            lo = 2 * gxp if q == 2 else 0
                            if kt < lo:
                                continue
                            st = kt == lo and not (q == 2 and gp)
                            nc.tensor.matmul(
                                ps[q], lhsT=xmb[:, kt], rhs=wbih_sb[:, kt, j, qi],
                                start=st, stop=False, skip_group_check=True,
                            )
                    for kt in range(KT):
                        last = kt == KT - 1
                        for qi, q in enumerate(bf_gates):
                            if q == 2 and kt < 2 * ghp:
                                continue
                            nc.tensor.matmul(
                                ps[q], lhsT=hmb[:, kt], rhs=wbhh_sb[:, kt, j, qi],
                                start=False, stop=last, skip_group_check=True,
                            )

                    g = [
                        apool.tile([128, NQ], f32, tag=f"g{q}", name=f"g{q}")
                        for q in range(4)
                    ]
                    # bias add on DVE (PSUM -> SBUF), then ACT in place with
                    # the fp8 gates' 1/ws de-scale folded into ACT's input
                    # scale (bias rows for those gates are pre-scaled by ws).
                    # Drain banks in readiness order — i/f/o close at the
                    # last fp8 h-wave and drain during the bf16 tail; g
                    # closes at group end and is touched late (tail wave)
                    # by the next group, so it drains last without stalling
                    # anyone (DVE runs in order: a premature g-add would
                    # gate every later drain on the group's final matmul).
                    for q in (0, 1, 3, 2):
                        nc.vector.tensor_add(out=g[q], in0=ps[q], in1=bias_sb[:, j, q])
                    for q in (0, 1, 3, 2):
                        nc.scalar.activation(
                            out=g[q], in_=g[q], func=ACT_FN[q],
                            scale=1.0 / gate_scale[q],
                        )

                    gi, gf, gg, go = g
                    nc.vector.tensor_mul(out=gi, in0=gi, in1=gg)   # sig(i)*tanh(g)
                    nc.vector.tensor_mul(out=gf, in0=gf, in1=c0t)  # sig(f)*c0
                    c1t = opool.tile([128, NQ], f32, tag="c1")
                    nc.vector.tensor_add(out=c1t, in0=gi, in1=gf)
                    tc1 = apool.tile([128, NQ], f32, tag="tc1")
                    nc.scalar.activation(out=tc1, in_=c1t, func=TANH)
                    h1t = opool.tile([128, NQ], f32, tag="h1")
                    nc.vector.tensor_mul(out=h1t, in0=go, in1=tc1)

                    nc.sync.dma_start(out=c1[rs, cs], in_=c1t)
                    nc.sync.dma_start(out=h1[rs, cs], in_=h1t)

            if repeat > 1:
                loop_cm.__exit__(None, None, None)

    _split_excess_waits(nc)
    if os.environ.get("LSTM_LDW_DEDUPE", "1") == "1":
        _dedupe_ldweights(nc)
    return nc


def _dedupe_ldweights(nc):
    """Remove an InstLdweights whose weights AP matches the previous
    InstLdweights on PE, with only InstMatmult in between — the PE array
    still holds those weights, so the reload is redundant. Only drops
    instructions with no semaphore waits/updates."""
    n = 0
    for f in nc.m.functions:
        for blk in f.blocks:
            prev_key = None
            keep = []
            for inst in blk.instructions:
                if getattr(inst, "engine", None) != mybir.EngineType.PE:
                    keep.append(inst)
                    continue
                tn = type(inst).__name__
                if tn == "InstLdweights":
                    w = inst.ins[0]
                    key = (
                        w.memref, w.offset, str(w.ap), str(w.dtype),
                        str(getattr(inst, "perf_mode", None)),
                    )
                    si = getattr(inst, "sync_info", None)
                    clean = si is None or (not si.on_wait and not si.on_update)
                    if key == prev_key and clean:
                        n += 1
                        continue  # drop it
                    prev_key = key
                elif tn != "InstMatmult":
                    prev_key = None
                keep.append(inst)
            blk.instructions[:] = keep
    return n


_NC = None
_NC_KEY = None


def _get_nc():
    global _NC, _NC_KEY
    key = (tuple(sorted(_fp8_gates())), _wscale(), _g_fp8_pairs())
    if _NC is None or _NC_KEY != key:
        _NC = _build_nc()
        _NC_KEY = key
    return _NC


def _prep_xT4(x, dt):
    """[B, 1024] fp32 -> [N_CORES][128, MT, KT, 128] per-core arrays."""
    v = x.reshape(N_CORES, MT, 128, KT, 128)  # [c, m, b, kt, p]
    v = v.transpose(0, 4, 1, 3, 2)            # [c, p, m, kt, b]
    v = v.astype(dt)
    return [np.ascontiguousarray(v[c]) for c in range(N_CORES)]


def _prep_w5(w, gates, dt, scales):
    """[4096, 1024] fp32, subset of gate blocks -> [128, KT, 2, nq, 512]."""
    v = w.reshape(4, 2, NQ, KT, 128)  # [q, j, s, kt, p]
    v = v[list(gates)] * np.asarray(scales)[:, None, None, None, None]
    v = v.transpose(4, 3, 1, 0, 2)    # [p, kt, j, nq, s]
    return np.ascontiguousarray(v.astype(dt))


def _prep_wg(w, n_kt, scale):
    """g-gate block's first n_kt k-tiles -> [128, n_kt, 2, 512] fp8."""
    v = w[2 * H:3 * H].reshape(2, NQ, KT, 128)  # [j, s, kt, p]
    v = v.transpose(3, 2, 0, 1)[:, :n_kt] * scale
    return np.ascontiguousarray(v.astype(FP8))


def _make_in_maps(input, h_0, c_0, W_ih, b_ih, W_hh, b_hh):
    fp8_gates = sorted(_fp8_gates())
    bf_gates = sorted(set(range(4)) - set(fp8_gates))
    ws = _wscale()
    gp = _g_fp8_pairs() if 2 in bf_gates else 0
    gxp, ghp = (gp + 1) // 2, gp // 2
    gate_scale = [ws if (q in fp8_gates or (q == 2 and gp)) else 1.0
                  for q in range(4)]

    x = np.asarray(input, dtype=np.float32)
    h0 = np.asarray(h_0, dtype=np.float32)
    c0 = np.asarray(c_0, dtype=np.float32)
    wih = np.asarray(W_ih, dtype=np.float32)
    whh = np.asarray(W_hh, dtype=np.float32)
    b = (np.asarray(b_ih, dtype=np.float32) + np.asarray(b_hh, dtype=np.float32))

    common = {}
    per_core = [dict() for _ in range(N_CORES)]
    if fp8_gates:
        xs8 = _prep_xT4(x, FP8)
        hs8 = _prep_xT4(h0, FP8)
        common["w8ih"] = _prep_w5(wih, fp8_gates, FP8, [ws] * len(fp8_gates))
        common["w8hh"] = _prep_w5(whh, fp8_gates, FP8, [ws] * len(fp8_gates))
        for c in range(N_CORES):
            per_core[c]["xT8"] = xs8[c]
            per_core[c]["hT8"] = hs8[c]
    if gxp:
        common["w8gih"] = _prep_wg(wih, 2 * gxp, ws)
    if ghp:
        common["w8ghh"] = _prep_wg(whh, 2 * ghp, ws)
    if bf_gates:
        xsb = _prep_xT4(x, BF16)
        hsb = _prep_xT4(h0, BF16)
        bscales = [gate_scale[q] for q in bf_gates]
        common["wbih"] = _prep_w5(wih, bf_gates, BF16, bscales)
        common["wbhh"] = _prep_w5(whh, bf_gates, BF16, bscales)
        for c in range(N_CORES):
            per_core[c]["xTb"] = xsb[c]
            per_core[c]["hTb"] = hsb[c]

    bq = b.reshape(4, 2, NQ).copy()  # [q, j, s]
    for q in range(4):
        bq[q] *= gate_scale[q]
    common["bjqs"] = np.ascontiguousarray(
        bq.transpose(1, 0, 2)[None].astype(np.float32)
    )  # [1, 2(j), 4(q), 512]

    c0s = c0.reshape(N_CORES, BL, H)
    return [
        {**common, "c0": np.ascontiguousarray(c0s[c]), **per_core[c]}
        for c in range(N_CORES)
    ]


def kernel(input, h_0, c_0, W_ih, b_ih, W_hh, b_hh):
    in_maps = _make_in_maps(input, h_0, c_0, W_ih, b_ih, W_hh, b_hh)
    nc = _get_nc()
    res = run_bass_kernel_spmd(nc, in_maps, core_ids=list(range(N_CORES)))
    h_1 = np.concatenate([res.results[c]["h1"] for c in range(N_CORES)], axis=0)
    c_1 = np.concatenate([res.results[c]["c1"] for c in range(N_CORES)], axis=0)
    return (h_1, c_1)


# revision 32
# speedup vs baseline: 1.1204x; 1.1204x over previous
"""Trainium2 Bass kernel for an LSTM cell (DPLSTMCell).

  gates = input @ W_ih^T + b_ih + h_0 @ W_hh^T + b_hh          [B, 4H]
  i, f, g, o = split(gates, 4)
  c_1 = sigmoid(f) * c_0 + sigmoid(i) * tanh(g)
  h_1 = sigmoid(o) * tanh(c_1)

B=16384, IN=H=1024. Data-parallel across 8 NeuronCores: each core gets a
2048-row batch shard; weights/biases are replicated.

Mixed-precision matmuls: the sigmoid gates (i, f, o) tolerate fp8 with the
2e-2 relative-error budget, so their GEMMs run as fp8e4m3 DoubleRow
(double-pumped, 2x bf16 throughput); the tanh cell gate g dominates the
quantization-error budget and stays mostly bf16 — 2 of its 8 DoubleRow
k-pairs are offloaded to fp8 (split-K), spending the remaining error
budget for ~14us (measured rel_err 1.811e-2 vs 1.554e-2 full-bf16-g).
Weights for fp8 GEMMs are pre-scaled by 8 on the host to lift them out of
e4m3's subnormal range (W is uniform +-1/32); the inverse 1/8 rides the
ACT engine's free input scale, with the bias pre-scaled by 8 so the DVE
bias-add stays a plain add. The g gate's bf16 weights are also scaled by
8 (free for bf16) so both its halves accumulate consistently in PSUM.

Two scheduling tricks worth ~37us together (519 -> 253us journey:
290 fp8 / 283 dedupe / 253 drain-order):
 - Weight DMA arrives as kt-slabs spanning all fp8 gates, so the
   sim-driven Tile scheduler interleaves the PSUM banks kt-major; the
   _dedupe_ldweights pass then drops the now-adjacent duplicate
   LdWeights (the PE array holds weights across matmuls).
 - The epilogue drains PSUM banks in readiness order (0,1,3 close at the
   last fp8 h-wave and drain during g's bf16 tail; g closes at group end
   and drains last). DVE executes in order, so draining g first would
   gate every bank's release on the group's final matmul and stall the
   next group's first wave.

Host-side prep (inside kernel()): x^T / h^T tiles and W^T in a
matmul-friendly 5D layout, cast per-gate to bf16 or fp8; b_ih+b_hh
combined (fp8 gates' entries x8). Device: bf16/fp8 matmuls with fp32 PSUM
accumulation plus an fp32 sigmoid/tanh epilogue.

Device layout per core (q8 indexes fp8 gates, qb bf16 gates):
  xT8/hT8 [128, MT, KT, 128] fp8  : xT8[p, m, kt, b] = x[m*128 + b, kt*128 + p]
  xTb/hTb [128, MT, KT, 128] bf16 : same indexing (only if bf16 gates exist)
  w8ih/w8hh [128, KT, 2, n8, 512] fp8  : [p, kt, j, q8, s] = 8*W[g(q8)*1024 + j*512 + s, kt*128 + p]
  wbih/wbhh [128, KT, 2, nb, 512] bf16 : [p, kt, j, qb, s] =   W[g(qb)*1024 + j*512 + s, kt*128 + p]
  bias [1, 2, 4, 512] fp32 : bias[0, j, q, s] = scale_q * (b_ih + b_hh)[q*1024 + j*512 + s]
  c0 / h1 / c1 [2048, 1024] fp32 natural.

Per batch-tile m (128 rows) and gate-column group j (512 of 1024 columns):
4 PSUM banks (i, f, g, o); fp8 banks accumulate 8 DoubleRow matmuls of
[128k x 2 x 128b]^T @ [128k x 2 x 512g], the bf16 bank 16 matmuls of
[128k x 128b]^T @ [128k x 512g]; the fp32 bias (DMA-broadcast across
partitions once) is added on DVE during the PSUM->SBUF move, then ACT
applies sigmoid/tanh (with the 1/8 de-scale for fp8 gates) and DVE forms
c_1 / h_1.
"""

import os
import sys

import numpy as np

for _p in ("/opt/trn_rl_repo", "/root/.axon_site/_ro/trn_rl_repo"):
    if os.path.isdir(_p) and _p not in sys.path:
        sys.path.append(_p)

import ml_dtypes  # noqa: E402

import concourse.bass as bass  # noqa: E402
import concourse.mybir as mybir  # noqa: E402
import concourse.tile as tile  # noqa: E402
from concourse.bass_utils import run_bass_kernel_spmd  # noqa: E402

N_CORES = 8
B = 16384
IN = 1024
H = 1024
BL = B // N_CORES  # 2048 rows per core
MT = BL // 128     # 16 batch tiles per core
KT = IN // 128     # 8 k-tiles
NQ = 512           # free dim per PSUM bank
BF16 = ml_dtypes.bfloat16
FP8 = ml_dtypes.float8_e4m3  # TRN2 fp8e4 (max 240)

_GATE_IDX = {"i": 0, "f": 1, "g": 2, "o": 3}


def _fp8_gates():
    """Set of gate indices computed in fp8 (default: i, f, o; g stays bf16)."""
    s = os.environ.get("LSTM_FP8_GATES", "i,f,o")
    return frozenset(_GATE_IDX[t] for t in s.split(",") if t)


def _g_fp8_pairs():
    """Number of the g gate's 8 DoubleRow k-pairs offloaded to fp8 (split
    evenly between the x and h sides; the rest stay bf16). Each pair moves
    1/8 of g's contraction to fp8: err grows ~sqrt(1 + 0.6*pairs),
    stream shrinks 6.8us/pair."""
    return int(os.environ.get("LSTM_G_FP8_PAIRS", "2"))


def _wscale():
    return float(os.environ.get("LSTM_WSCALE", "8"))


# The walrus in this container only accepts one sync-wait command per
# instruction; Tile emits instructions (notably the final drain) with more.
_MAX_WAITS_PER_INST = 1


def _split_excess_waits(nc, cap=_MAX_WAITS_PER_INST):
    """Move excess sem-waits onto NoOps inserted ahead of the instruction
    (same engine). Waits are AND-conditions on monotonically increasing
    semaphores, so satisfying them one-by-one is equivalent."""
    for f in nc.m.functions:
        for blk in f.blocks:
            new_insts = []
            for inst in blk.instructions:
                si = getattr(inst, "sync_info", None)
                if si is not None and si.on_wait and len(si.on_wait) > cap:
                    waits = list(si.on_wait)
                    extra, keep = waits[:-cap], waits[-cap:]
                    while extra:
                        chunk, extra = extra[:cap], extra[cap:]
                        new_insts.append(
                            mybir.InstNoOp(
                                name=nc.get_next_instruction_name(),
                                sync_info=mybir.SyncInfo(on_wait=chunk, on_update=[]),
                                bass_nofuse=True,
                                engine=inst.engine,
                            )
                        )
                    inst.sync_info = mybir.SyncInfo(
                        on_wait=keep, on_update=list(si.on_update or [])
                    )
                new_insts.append(inst)
            blk.instructions[:] = new_insts


def _build_nc(repeat=None):
    """repeat>1 wraps the whole body in a hardware loop — benchmarking only
    (outputs are simply rewritten each iteration)."""
    if repeat is None:
        repeat = int(os.environ.get("LSTM_BENCH_REPEAT", "1"))
    fp8_gates = sorted(_fp8_gates())
    bf_gates = sorted(set(range(4)) - set(fp8_gates))
    n8, nb = len(fp8_gates), len(bf_gates)
    ws = _wscale()
    gp = _g_fp8_pairs() if (2 in bf_gates and n8) else 0
    gxp, ghp = (gp + 1) // 2, gp // 2  # g's fp8 k-pairs on the x / h side
    # per-gate pre-activation scale (weights+bias scaled up by s, ACT
    # de-scales by 1/s): fp8 gates use ws to clear e4m3's subnormal range;
    # a split-K g uses ws on BOTH its fp8 and bf16 parts so the PSUM
    # accumulation is consistent (bf16 is scale-invariant, so this is free)
    gate_scale = [ws if (q in fp8_gates or (q == 2 and gp)) else 1.0
                  for q in range(4)]

    nc = bass.Bass()
    f32 = mybir.dt.float32
    bf16 = mybir.dt.bfloat16
    fp8 = mybir.dt.float8e4
    DR = mybir.MatmulPerfMode.DoubleRow
    SIG = mybir.ActivationFunctionType.Sigmoid
    TANH = mybir.ActivationFunctionType.Tanh
    ACT_FN = {0: SIG, 1: SIG, 2: TANH, 3: SIG}

    c0 = nc.declare_dram_parameter("c0", [BL, H], f32, isOutput=False)
    if n8:
        xT8 = nc.declare_dram_parameter("xT8", [128, MT, KT, 128], fp8, isOutput=False)
        hT8 = nc.declare_dram_parameter("hT8", [128, MT, KT, 128], fp8, isOutput=False)
        w8ih = nc.declare_dram_parameter("w8ih", [128, KT, 2, n8, NQ], fp8, isOutput=False)
        w8hh = nc.declare_dram_parameter("w8hh", [128, KT, 2, n8, NQ], fp8, isOutput=False)
    if gxp:
        w8gih = nc.declare_dram_parameter("w8gih", [128, 2 * gxp, 2, NQ], fp8, isOutput=False)
    if ghp:
        w8ghh = nc.declare_dram_parameter("w8ghh", [128, 2 * ghp, 2, NQ], fp8, isOutput=False)
    if nb:
        xTb = nc.declare_dram_parameter("xTb", [128, MT, KT, 128], bf16, isOutput=False)
        hTb = nc.declare_dram_parameter("hTb", [128, MT, KT, 128], bf16, isOutput=False)
        wbih = nc.declare_dram_parameter("wbih", [128, KT, 2, nb, NQ], bf16, isOutput=False)
        wbhh = nc.declare_dram_parameter("wbhh", [128, KT, 2, nb, NQ], bf16, isOutput=False)
    bjqs = nc.declare_dram_parameter("bjqs", [1, 2, 4, NQ], f32, isOutput=False)
    h1 = nc.declare_dram_parameter("h1", [BL, H], f32, isOutput=True)
    c1 = nc.declare_dram_parameter("c1", [BL, H], f32, isOutput=True)

    with tile.TileContext(nc) as tc:
        with (
            tc.tile_pool(name="w", bufs=1) as wpool,
            tc.tile_pool(name="xh", bufs=4) as xhpool,
            tc.tile_pool(name="cc", bufs=4) as cpool,
            tc.tile_pool(name="act", bufs=2) as apool,
            tc.tile_pool(name="outp", bufs=4) as opool,
            tc.tile_pool(name="ps", bufs=8, space="PSUM") as pspool,
        ):
            if n8:
                w8ih_sb = wpool.tile([128, KT, 2, n8, NQ], fp8)
                w8hh_sb = wpool.tile([128, KT, 2, n8, NQ], fp8)
            if gxp:
                w8gih_sb = wpool.tile([128, 2 * gxp, 2, NQ], fp8)
            if ghp:
                w8ghh_sb = wpool.tile([128, 2 * ghp, 2, NQ], fp8)
            if nb:
                wbih_sb = wpool.tile([128, KT, 2, nb, NQ], bf16)
                wbhh_sb = wpool.tile([128, KT, 2, nb, NQ], bf16)
            bias_sb = wpool.tile([128, 2, 4, NQ], f32)

            if repeat > 1:
                loop_cm = tc.For_i(0, repeat, 1)
                loop_cm.__enter__()

            # Weights on the SP HWDGE queue in ~consumption order, as
            # kt-slabs spanning ALL gates of a dtype class: with every
            # bank's kt-wave arriving together, the greedy scheduler
            # interleaves banks kt-major (priority = source order), which
            # makes the per-pair LdWeights shareable across banks — the
            # dedupe pass then drops the redundant reloads. x/h/c0 loads
            # go on the ACT queue; outputs on SP after weights.
            for j in range(2):
                if n8:
                    for kh in range(2):
                        ks = slice(kh * 4, (kh + 1) * 4)
                        nc.sync.dma_start(out=w8ih_sb[:, ks, j], in_=w8ih[:, ks, j])
                        if kh == 0 and gxp:
                            nc.sync.dma_start(out=w8gih_sb[:, :, j], in_=w8gih[:, :, j])
                    for kh in range(2):
                        ks = slice(kh * 4, (kh + 1) * 4)
                        nc.sync.dma_start(out=w8hh_sb[:, ks, j], in_=w8hh[:, ks, j])
                        if kh == 0 and ghp:
                            nc.sync.dma_start(out=w8ghh_sb[:, :, j], in_=w8ghh[:, :, j])
                if nb:
                    for w_sb, w_dr, gskip in (
                        (wbih_sb, wbih, 2 * gxp), (wbhh_sb, wbhh, 2 * ghp)
                    ):
                        for qi, q in enumerate(bf_gates):
                            lo = gskip if q == 2 else 0
                            for kh in range(2):
                                ks = slice(max(kh * 4, lo), (kh + 1) * 4)
                                if ks.start >= ks.stop:
                                    continue
                                nc.sync.dma_start(
                                    out=w_sb[:, ks, j, qi], in_=w_dr[:, ks, j, qi]
                                )

            for m in range(MT):
                if n8:
                    xm8 = xhpool.tile([128, KT, 128], fp8, tag="xm8")
                    hm8 = xhpool.tile([128, KT, 128], fp8, tag="hm8")
                    nc.scalar.dma_start(out=xm8, in_=xT8[:, m])
                    nc.scalar.dma_start(out=hm8, in_=hT8[:, m])
                if nb:
                    xmb = xhpool.tile([128, KT, 128], bf16, tag="xmb")
                    hmb = xhpool.tile([128, KT, 128], bf16, tag="hmb")
                    nc.scalar.dma_start(out=xmb, in_=xTb[:, m])
                    nc.scalar.dma_start(out=hmb, in_=hTb[:, m])
                if m == 0:
                    # bias isn't needed until the first matmul group finishes;
                    # keep it behind the first x/h tiles on the ACT queue.
                    bj_ap = bjqs[:]
                    bias_bcast = bass.AP(
                        tensor=bj_ap.tensor,
                        offset=bj_ap.offset,
                        ap=[[0, 128]] + list(bj_ap.ap[1:]),
                    )
                    nc.scalar.dma_start(out=bias_sb, in_=bias_bcast)
                for j in range(2):
                    cs = slice(j * NQ, (j + 1) * NQ)
                    rs = slice(m * 128, (m + 1) * 128)

                    c0t = cpool.tile([128, NQ], f32, tag="c0")
                    nc.scalar.dma_start(out=c0t, in_=c0[rs, cs])

                    ps = [
                        pspool.tile([128, NQ], f32, tag="ps", name=f"ps{q}")
                        for q in range(4)
                    ]
                    # fp8 gates: DoubleRow pairs two k-tiles per matmul. The
                    # g gate's split-K fp8 pairs ride the first kt-waves and
                    # share the interleaved chains' LdWeights.
                    for t in range(KT // 2):
                        kp = slice(2 * t, 2 * t + 2)
                        for qi, q in enumerate(fp8_gates):
                            nc.tensor.matmul(
                                ps[q], lhsT=xm8[:, kp], rhs=w8ih_sb[:, kp, j, qi],
                                start=(t == 0), stop=False,
                                perf_mode=DR, skip_group_check=True,
                            )
                        if t < gxp:
                            nc.tensor.matmul(
                                ps[2], lhsT=xm8[:, kp],
                                rhs=w8gih_sb[:, 2 * t:2 * t + 2, j],
                                start=(t == 0), stop=False,
                                perf_mode=DR, skip_group_check=True,
                            )
                    for t in range(KT // 2):
                        kp = slice(2 * t, 2 * t + 2)
                        last = t == KT // 2 - 1
                        for qi, q in enumerate(fp8_gates):
                            nc.tensor.matmul(
                                ps[q], lhsT=hm8[:, kp], rhs=w8hh_sb[:, kp, j, qi],
                                start=False, stop=last,
                                perf_mode=DR, skip_group_check=True,
                            )
                        if t < ghp:
                            nc.tensor.matmul(
                                ps[2], lhsT=hm8[:, kp],
                                rhs=w8ghh_sb[:, 2 * t:2 * t + 2, j],
                                start=False, stop=False,
                                perf_mode=DR, skip_group_check=True,
                            )
                    # bf16 gates: one k-tile per matmul (g skips its fp8 kts).
                    for kt in range(KT):
                        for qi, q in enumerate(bf_gates):
                            lo = 2 * gxp if q == 2 else 0
                            if kt < lo:
                                continue
                            st = kt == lo and not (q == 2 and gp)
                            nc.tensor.matmul(
                                ps[q], lhsT=xmb[:, kt], rhs=wbih_sb[:, kt, j, qi],
                                start=st, stop=False, skip_group_check=True,
                            )
                    for kt in range(KT):
                        last = kt == KT - 1
                        for qi, q in enumerate(bf_gates):
                            if q == 2 and kt < 2 * ghp:
                                continue
                            nc.tensor.matmul(
                                ps[q], lhsT=hmb[:, kt], rhs=wbhh_sb[:, kt, j, qi],
                                start=False, stop=last, skip_group_check=True,
                            )

                    g = [
                        apool.tile([128, NQ], f32, tag=f"g{q}", name=f"g{q}")
                        for q in range(4)
                    ]
                    # bias add on DVE (PSUM -> SBUF), then ACT in place with
                    # the fp8 gates' 1/ws de-scale folded into ACT's input
                    # scale (bias rows for those gates are pre-scaled by ws).
                    # Drain banks in readiness order — i/f/o close at the
                    # last fp8 h-wave and drain during g's bf16 tail; g
                    # closes at the group's final matmul and drains last.
                    # DVE runs in order: draining g earlier would gate the
                    # other banks' release on that final matmul and stall
                    # the next group's first wave (measured +28us).
                    for q in (0, 1, 3, 2):
                        nc.vector.tensor_add(out=g[q], in0=ps[q], in1=bias_sb[:, j, q])
                    for q in (0, 1, 3, 2):
                        nc.scalar.activation(
                            out=g[q], in_=g[q], func=ACT_FN[q],
                            scale=1.0 / gate_scale[q],
                        )

                    gi, gf, gg, go = g
                    nc.vector.tensor_mul(out=gi, in0=gi, in1=gg)   # sig(i)*tanh(g)
                    nc.vector.tensor_mul(out=gf, in0=gf, in1=c0t)  # sig(f)*c0
                    c1t = opool.tile([128, NQ], f32, tag="c1")
                    nc.vector.tensor_add(out=c1t, in0=gi, in1=gf)
                    tc1 = apool.tile([128, NQ], f32, tag="tc1")
                    nc.scalar.activation(out=tc1, in_=c1t, func=TANH)
                    h1t = opool.tile([128, NQ], f32, tag="h1")
                    nc.vector.tensor_mul(out=h1t, in0=go, in1=tc1)

                    nc.sync.dma_start(out=c1[rs, cs], in_=c1t)
                    nc.sync.dma_start(out=h1[rs, cs], in_=h1t)

            if repeat > 1:
                loop_cm.__exit__(None, None, None)

    _split_excess_waits(nc)
    if os.environ.get("LSTM_LDW_DEDUPE", "1") == "1":
        _dedupe_ldweights(nc)
    return nc


def _dedupe_ldweights(nc):
    """Remove an InstLdweights whose weights AP matches the previous
    InstLdweights on PE, with only InstMatmult in between — the PE array
    still holds those weights, so the reload is redundant. Only drops
    instructions with no semaphore waits/updates."""
    n = 0
    for f in nc.m.functions:
        for blk in f.blocks:
            prev_key = None
            keep = []
            for inst in blk.instructions:
                if getattr(inst, "engine", None) != mybir.EngineType.PE:
                    keep.append(inst)
                    continue
                tn = type(inst).__name__
                if tn == "InstLdweights":
                    w = inst.ins[0]
                    key = (
                        w.memref, w.offset, str(w.ap), str(w.dtype),
                        str(getattr(inst, "perf_mode", None)),
                    )
                    si = getattr(inst, "sync_info", None)
                    clean = si is None or (not si.on_wait and not si.on_update)
                    if key == prev_key and clean:
                        n += 1
                        continue  # drop it
                    prev_key = key
                elif tn != "InstMatmult":
                    prev_key = None
                keep.append(inst)
            blk.instructions[:] = keep
    return n


_NC = None
_NC_KEY = None


def _get_nc():
    global _NC, _NC_KEY
    key = (tuple(sorted(_fp8_gates())), _wscale(), _g_fp8_pairs())
    if _NC is None or _NC_KEY != key:
        _NC = _build_nc()
        _NC_KEY = key
    return _NC


def _prep_xT4(x, dt):
    """[B, 1024] fp32 -> [N_CORES][128, MT, KT, 128] per-core arrays."""
    v = x.reshape(N_CORES, MT, 128, KT, 128)  # [c, m, b, kt, p]
    v = v.transpose(0, 4, 1, 3, 2)            # [c, p, m, kt, b]
    v = v.astype(dt)
    return [np.ascontiguousarray(v[c]) for c in range(N_CORES)]


def _prep_w5(w, gates, dt, scales):
    """[4096, 1024] fp32, subset of gate blocks -> [128, KT, 2, nq, 512]."""
    v = w.reshape(4, 2, NQ, KT, 128)  # [q, j, s, kt, p]
    v = v[list(gates)] * np.asarray(scales)[:, None, None, None, None]
    v = v.transpose(4, 3, 1, 0, 2)    # [p, kt, j, nq, s]
    return np.ascontiguousarray(v.astype(dt))


def _prep_wg(w, n_kt, scale):
    """g-gate block's first n_kt k-tiles -> [128, n_kt, 2, 512] fp8."""
    v = w[2 * H:3 * H].reshape(2, NQ, KT, 128)  # [j, s, kt, p]
    v = v.transpose(3, 2, 0, 1)[:, :n_kt] * scale
    return np.ascontiguousarray(v.astype(FP8))


def _make_in_maps(input, h_0, c_0, W_ih, b_ih, W_hh, b_hh):
    fp8_gates = sorted(_fp8_gates())
    bf_gates = sorted(set(range(4)) - set(fp8_gates))
    ws = _wscale()
    gp = _g_fp8_pairs() if (2 in bf_gates and fp8_gates) else 0
    gxp, ghp = (gp + 1) // 2, gp // 2
    gate_scale = [ws if (q in fp8_gates or (q == 2 and gp)) else 1.0
                  for q in range(4)]

    x = np.asarray(input, dtype=np.float32)
    h0 = np.asarray(h_0, dtype=np.float32)
    c0 = np.asarray(c_0, dtype=np.float32)
    wih = np.asarray(W_ih, dtype=np.float32)
    whh = np.asarray(W_hh, dtype=np.float32)
    b = (np.asarray(b_ih, dtype=np.float32) + np.asarray(b_hh, dtype=np.float32))

    common = {}
    per_core = [dict() for _ in range(N_CORES)]
    if fp8_gates:
        xs8 = _prep_xT4(x, FP8)
        hs8 = _prep_xT4(h0, FP8)
        common["w8ih"] = _prep_w5(wih, fp8_gates, FP8, [ws] * len(fp8_gates))
        common["w8hh"] = _prep_w5(whh, fp8_gates, FP8, [ws] * len(fp8_gates))
        for c in range(N_CORES):
            per_core[c]["xT8"] = xs8[c]
            per_core[c]["hT8"] = hs8[c]
    if gxp:
        common["w8gih"] = _prep_wg(wih, 2 * gxp, ws)
    if ghp:
        common["w8ghh"] = _prep_wg(whh, 2 * ghp, ws)
    if bf_gates:
        xsb = _prep_xT4(x, BF16)
        hsb = _prep_xT4(h0, BF16)
        bscales = [gate_scale[q] for q in bf_gates]
        common["wbih"] = _prep_w5(wih, bf_gates, BF16, bscales)
        common["wbhh"] = _prep_w5(whh, bf_gates, BF16, bscales)
        for c in range(N_CORES):
            per_core[c]["xTb"] = xsb[c]
            per_core[c]["hTb"] = hsb[c]

    bq = b.reshape(4, 2, NQ).copy()  # [q, j, s]
    for q in range(4):
        bq[q] *= gate_scale[q]
    common["bjqs"] = np.ascontiguousarray(
        bq.transpose(1, 0, 2)[None].astype(np.float32)
    )  # [1, 2(j), 4(q), 512]

    c0s = c0.reshape(N_CORES, BL, H)
    return [
        {**common, "c0": np.ascontiguousarray(c0s[c]), **per_core[c]}
        for c in range(N_CORES)
    ]


def kernel(input, h_0, c_0, W_ih, b_ih, W_hh, b_hh):
    in_maps = _make_in_maps(input, h_0, c_0, W_ih, b_ih, W_hh, b_hh)
    nc = _get_nc()
    res = run_bass_kernel_spmd(nc, in_maps, core_ids=list(range(N_CORES)))
    h_1 = np.concatenate([res.results[c]["h1"] for c in range(N_CORES)], axis=0)
    c_1 = np.concatenate([res.results[c]["c1"] for c in range(N_CORES)], axis=0)
    return (h_1, c_1)
